# revision 1
# baseline (speedup 1.0000x reference)
"""Trainium2 Bass kernel for nn_Attention_34033320854122.

Dense transformer attention block: QKV proj -> causal depthwise conv+SiLU ->
per-head RMSNorm -> partial RoPE -> causal attention -> output projection.

Sharding: tensor-parallel over the 16 heads across 8 NeuronCores (2 heads =
256 channels per core). Each core computes q/k/v for its channels (full
contraction over D), runs attention for its 2 heads, and produces a partial
output projection (outT_partial = Wo[:, cols] @ attn_cols^T). The host sums
the 8 partials and transposes.

Notes on fidelity to the reference:
- The reference negates the rotated RoPE sub-dim of BOTH q and k
  (return concat([-x_rot, x_pass])). The negation cancels exactly in
  q . k, so it is skipped.
- softmax is computed without max-subtraction: scores are O(1)-bounded
  (RMS-normed q/k, scale 1/sqrt(128)), far from fp32 exp overflow.

Matmuls run in float32r (PE's reduced-precision fp32 mode, ~13-bit
mantissa, full throughput at moving-dim >= 256). Raw fp32 bytes DMA'd into
f32r tiles behave bit-identically to the gpsimd cast-DMA path (verified).
"""

import ml_dtypes
import numpy as np

import concourse.bacc as bacc
import concourse.tile as tile
import concourse.mybir as mybir
from concourse import bass_utils
from concourse.masks import make_identity

# Problem shape (hardcoded per contract)
B, T, D = 1, 2048, 2048
H, HD = 16, 128
RD = 64
KCONV = 4
EPS = 1e-5
NCORES = 8
CPC = D // NCORES      # channels per core = 256
MPC = CPC // HD        # head tiles per core = 2
NT = 512               # free-dim tile for matmuls
NQ = T // NT           # 4 q tiles
NKC = T // HD          # 16 key chunks of 128
KD = D // 128          # 16 contraction chunks
PAD = KCONV - 1        # causal conv history

F32 = mybir.dt.float32
F32R = mybir.dt.float32r
BF16 = mybir.dt.bfloat16

_COMPILED = None


def _build():
    nc = bacc.Bacc("TRN2", target_bir_lowering=False, debug=False,
                   num_devices=NCORES)

    d = {}
    d["xT"] = nc.dram_tensor("xT", (D, T), BF16, kind="ExternalInput").ap()
    d["wqT"] = nc.dram_tensor("wqT", (D, CPC), BF16, kind="ExternalInput").ap()
    d["wkT"] = nc.dram_tensor("wkT", (D, CPC), BF16, kind="ExternalInput").ap()
    d["wvT"] = nc.dram_tensor("wvT", (D, CPC), BF16, kind="ExternalInput").ap()
    d["woT"] = nc.dram_tensor("woT", (CPC, D), F32R, kind="ExternalInput").ap()
    # trig: rows 0:64 = cos^T, rows 64:128 = sign-folded sin^T
    d["trig"] = nc.dram_tensor("trig", (128, T), F32, kind="ExternalInput").ap()
    # conv weights packed [128, proj(3), m(2), tap(4)]
    d["convw"] = nc.dram_tensor("convw", (128, 3, 2, KCONV), F32,
                                kind="ExternalInput").ap()
    # per-head norm weights [128, 2] (q, k)
    d["normw"] = nc.dram_tensor("normw", (128, 2), F32, kind="ExternalInput").ap()
    # causal mask strip [128, 896]: mask[kl, c] = 1.0 if kl <= c - 384
    d["maskb"] = nc.dram_tensor("maskb", (128, 896), F32R,
                                kind="ExternalInput").ap()
    outT = nc.dram_tensor("outT", (D, T), F32, kind="ExternalOutput").ap()

    inv_sqrt_hd = 1.0 / np.sqrt(HD)

    with tile.TileContext(nc) as tc:
        with (
            tc.tile_pool(name="consts", bufs=1) as consts,
            tc.tile_pool(name="raw", bufs=1) as rawp,
            tc.tile_pool(name="wqkv", bufs=1) as wqkvp,
            tc.tile_pool(name="final", bufs=1) as finalp,
            tc.tile_pool(name="xblk", bufs=2) as xp,
            tc.tile_pool(name="scratch", bufs=2) as scr,
            tc.tile_pool(name="exp", bufs=3) as expp,
            tc.tile_pool(name="attn", bufs=3) as attnp,
            tc.tile_pool(name="ostage", bufs=3) as ostp,
            tc.tile_pool(name="wo", bufs=2) as wop,
            tc.tile_pool(name="psacc", bufs=4, space="PSUM") as psacc,
            tc.tile_pool(name="pssum", bufs=1, space="PSUM") as pssum,
            tc.tile_pool(name="pssm", bufs=3, space="PSUM") as pssm,
        ):
            # ---- constants ----
            # cosT rows 0:64; ssin2: rows 0:32 = +sin, rows 32:64 = -sin
            # (both tiles base-partition 0 so 2-input DVE ops stay aligned)
            cosT_t = consts.tile([64, T], F32)
            nc.scalar.dma_start(cosT_t, d["trig"][0:64])
            ssin2_t = consts.tile([64, T], F32)
            nc.scalar.dma_start(ssin2_t, d["trig"][64:128])
            convw_t = consts.tile([128, 3, 2, KCONV], F32)
            nc.sync.dma_start(convw_t, d["convw"])
            normw_t = consts.tile([128, 2], F32)
            nc.sync.dma_start(normw_t, d["normw"])
            mask_t = consts.tile([128, 896], F32R)
            nc.scalar.dma_start(mask_t, d["maskb"])
            ones_f = consts.tile([128, 1], F32)
            nc.vector.memset(ones_f, 1.0)
            ones_hd = consts.tile([128, 1], F32R)   # lhsT for partition sums
            nc.vector.tensor_copy(ones_hd, ones_f)
            ones_1f = consts.tile([1, 128], F32)
            nc.vector.memset(ones_1f, 1.0)
            ones_1 = consts.tile([1, 128], F32R)    # lhsT for bcast over parts
            nc.vector.tensor_copy(ones_1, ones_1f)
            ident_f = consts.tile([128, 128], F32)
            make_identity(nc, ident_f)
            ident = consts.tile([128, 128], F32R)
            nc.vector.tensor_copy(ident, ident_f)
            eps_t = consts.tile([1, 1], F32)
            nc.vector.memset(eps_t, EPS)

            # ---- persistent buffers ----
            # raw (pre-conv) projections, padded by PAD zero cols at left
            rawq = rawp.tile([128, MPC, T + PAD], BF16)
            rawk = rawp.tile([128, MPC, T + PAD], BF16)
            rawv = rawp.tile([128, MPC, T + PAD], BF16)
            for r in (rawq, rawk, rawv):
                nc.vector.memset(r[:, :, 0:PAD], 0.0)
            # final q/k in head-transposed layout [HD, m, T]
            qfT = finalp.tile([128, MPC, T], F32R)
            kfT = finalp.tile([128, MPC, T], F32R)
            # v in natural layout per key-chunk: [t(128), m, chunk, HD]
            vtr = finalp.tile([128, MPC, NKC, HD], F32R)

            # =============== Phase bodies (emitted software-pipelined) ====
            # A(t): QKV projection for q-tile t.  B(s): conv/silu/rms/rope
            # for slice s (needs A(s) only, thanks to the causal pad).
            # C(t): attention + output projection for q-tile t (needs B(<=t)).
            w_all = wqkvp.tile([128, KD, 3, CPC], BF16)
            raws = (rawq, rawk, rawv)
            groups = [[(0, 0), (0, 1), (1, 0)], [(1, 1), (2, 0), (2, 1)]]

            def phaseA(tq, first=False):
                xb = xp.tile([128, KD, NT], BF16, name="xb")
                for k in range(KD):
                    if first:  # interleave weight-chunk loads with x(0)
                        for pi, wd in enumerate((d["wqT"], d["wkT"],
                                                 d["wvT"])):
                            deng = nc.sync if (k * 3 + pi) % 2 == 0 \
                                else nc.scalar
                            deng.dma_start(
                                w_all[:, k, pi, :],
                                wd[k * 128:(k + 1) * 128, :])
                    deng = nc.sync if k % 2 == 0 else nc.scalar
                    deng.dma_start(
                        xb[:, k, :],
                        d["xT"][k * 128:(k + 1) * 128,
                                tq * NT:(tq + 1) * NT],
                    )
                for grp in groups:
                    pst = [psacc.tile([128, NT], F32, tag="acc",
                                      name=f"acc{gi}")
                           for gi in range(3)]
                    for k in range(KD):
                        for gi, (pi, m) in enumerate(grp):
                            nc.tensor.matmul(
                                pst[gi],
                                w_all[:, k, pi, m * 128:(m + 1) * 128],
                                xb[:, k, :],
                                start=(k == 0),
                                stop=(k == KD - 1),
                            )
                    for gi, (pi, m) in enumerate(grp):
                        dst = raws[pi][:, m,
                                       PAD + tq * NT:PAD + (tq + 1) * NT]
                        nc.vector.tensor_copy(dst, pst[gi])

            def conv4(raw, pi, m, s):
                """4-tap causal depthwise conv on a 512-slice -> f32 scratch."""
                base = s * NT
                t0 = scr.tile([128, NT], F32, tag="cvA", name="cv0")
                nc.vector.tensor_scalar_mul(
                    t0, raw[:, m, base:base + NT], convw_t[:, pi, m, 0:1]
                )
                for j in (1, 2, 3):
                    t1 = scr.tile([128, NT], F32, tag=("cvB", "cvA")[j % 2],
                                  name="cvj")
                    nc.vector.scalar_tensor_tensor(
                        t1, raw[:, m, base + j:base + j + NT],
                        convw_t[:, pi, m, j:j + 1], t0,
                        mybir.AluOpType.mult, mybir.AluOpType.add,
                    )
                    t0 = t1
                return t0

            def phaseB(s):
                sl = slice(s * NT, (s + 1) * NT)
                for m in range(MPC):
                    # ---- q and k: conv, silu, rms-norm, rope ----
                    for pi, raw, fin, nwi in ((0, rawq, qfT, 0),
                                              (1, rawk, kfT, 1)):
                        cv = conv4(raw, pi, m, s)
                        sv = scr.tile([128, NT], F32, tag="silu")
                        nc.scalar.activation(
                            sv, cv, mybir.ActivationFunctionType.Silu)
                        sq = scr.tile([128, NT], F32R, tag="sq")
                        nc.scalar.activation(
                            sq, sv, mybir.ActivationFunctionType.Square)
                        ps_ss = pssm.tile([1, NT], F32, tag="sm")
                        nc.tensor.matmul(ps_ss, ones_hd, sq,
                                         start=True, stop=True)
                        rstd = scr.tile([1, NT], F32, tag="rst", name="rstd")
                        nc.scalar.activation(
                            rstd, ps_ss, mybir.ActivationFunctionType.Sqrt,
                            scale=1.0 / HD, bias=eps_t)
                        rr = scr.tile([1, NT], F32, tag="rst", name="rr")
                        nc.vector.reciprocal_approx_fast(rr, rstd)
                        ps_rb = pssm.tile([128, NT], F32, tag="sm")
                        nc.tensor.matmul(ps_rb, ones_1f, rr,
                                         start=True, stop=True)
                        qn = sv
                        nc.vector.scalar_tensor_tensor(
                            qn, sv, normw_t[:, nwi:nwi + 1], ps_rb,
                            mybir.AluOpType.mult, mybir.AluOpType.mult,
                        )
                        # rope rows 0:RD (pass-through rows RD:128):
                        # rot2[:,0] = qn_rot*cos; rot2[:,1] = rotate_half(qn)
                        # * sign-folded sin via output-offset muls.
                        rot2 = scr.tile([64, 2, NT], F32, tag="rot2")
                        nc.gpsimd.tensor_mul(rot2[0:32, 1, :], qn[32:64],
                                             ssin2_t[32:64, sl])
                        nc.gpsimd.tensor_mul(rot2[32:64, 1, :], qn[0:32],
                                             ssin2_t[0:32, sl])
                        nc.vector.tensor_mul(rot2[:, 0, :], qn[0:RD],
                                             cosT_t[:, sl])
                        nc.gpsimd.tensor_add(fin[0:RD, m, sl], rot2[:, 0, :],
                                             rot2[:, 1, :])
                        nc.scalar.copy(fin[RD:128, m, sl], qn[RD:128])
                    # ---- v: conv, silu, transpose to natural layout ----
                    cv = conv4(rawv, 2, m, s)
                    vv = scr.tile([128, NT], F32R, tag="gvB", name="vv")
                    nc.scalar.activation(
                        vv, cv, mybir.ActivationFunctionType.Silu)
                    ps_tr = pssm.tile([128, NT], F32R, tag="sm")
                    for sub in range(NT // 128):
                        nc.tensor.transpose(
                            ps_tr[:, sub * 128:(sub + 1) * 128],
                            vv[:, sub * 128:(sub + 1) * 128], ident)
                    nc.scalar.copy(
                        vtr[:, m, s * (NT // 128):(s + 1) * (NT // 128), :],
                        ps_tr.rearrange("p (s h) -> p s h", h=128))

            def phaseC(tq):
                qsl = slice(tq * NT, (tq + 1) * NT)
                attn_m = []
                for m in range(MPC):
                    nch = 4 * tq + 4
                    ps_attn = psacc.tile([128, NT], F32, tag="acc",
                                         name="ps_attn")
                    ps_sum = pssum.tile([1, NT], F32, tag="sum1",
                                        name="ps_sum")

                    def qk(tk):
                        ps_s = pssm.tile([128, NT], F32, tag="sm",
                                         name="ps_s")
                        nc.tensor.matmul(
                            ps_s, kfT[:, m, tk * 128:(tk + 1) * 128],
                            qfT[:, m, qsl], start=True, stop=True)
                        e = expp.tile([128, NT], F32R, tag="e", name="e")
                        nc.scalar.activation(
                            e, ps_s, mybir.ActivationFunctionType.Exp,
                            scale=inv_sqrt_hd)
                        dd = tk * 128 - tq * NT
                        if dd >= 0:  # diagonal chunk: causal mask
                            nc.vector.tensor_mul(
                                e, e, mask_t[:, 384 - dd:896 - dd])
                        return e

                    # software-pipeline QK ahead of PV by two chunks
                    epipe = [qk(t) for t in range(min(2, nch))]
                    for tk in range(nch):
                        if tk + 2 < nch:
                            epipe.append(qk(tk + 2))
                        e = epipe.pop(0)
                        nc.tensor.matmul(
                            ps_attn, vtr[:, m, tk, :], e,
                            start=(tk == 0), stop=(tk == nch - 1))
                        nc.tensor.matmul(
                            ps_sum, ones_hd, e,
                            start=(tk == 0), stop=(tk == nch - 1))
                    # normalize: attn^T *= 1/sumexp (broadcast over parts)
                    rr = scr.tile([1, NT], F32, tag="rst", name="rrs")
                    nc.vector.reciprocal_approx_fast(rr, ps_sum)
                    ps_rb = pssm.tile([128, NT], F32, tag="sm", name="ps_rb")
                    nc.tensor.matmul(ps_rb, ones_1f, rr, start=True,
                                     stop=True)
                    rb = scr.tile([128, NT], F32, tag="rbs")
                    nc.scalar.copy(rb, ps_rb)
                    am = attnp.tile([128, NT], F32R, tag="am", name="am")
                    nc.vector.tensor_mul(am, ps_attn, rb)
                    attn_m.append(am)
                # output projection for this q tile (wo prefetch 2 ahead)
                def wo_load(i):
                    wo_ch = wop.tile([128, 2, 128], F32R, tag="wo",
                                     name="wo_ch")
                    nc.sync.dma_start(
                        wo_ch,
                        d["woT"][:, i * 128:(i + 1) * 128].rearrange(
                            "(j p) n -> p j n", p=128))
                    return wo_ch
                wopipe = [wo_load(0), wo_load(1)]
                for i in range(D // 128):
                    if i + 2 < D // 128:
                        wopipe.append(wo_load(i + 2))
                    wo_ch = wopipe.pop(0)
                    ps_o = psacc.tile([128, NT], F32, tag="acc", name="ps_o")
                    for j in range(MPC):
                        nc.tensor.matmul(ps_o, wo_ch[:, j, :], attn_m[j],
                                         start=(j == 0), stop=(j == MPC - 1))
                    ost = ostp.tile([128, NT], F32, tag="ost", name="ost")
                    nc.vector.tensor_copy(ost, ps_o)
                    nc.sync.dma_start(outT[i * 128:(i + 1) * 128, qsl], ost)

            # pipelined emission: A two tiles ahead of B/C
            phaseA(0, first=True)
            phaseA(1)
            for t in range(NQ):
                phaseB(t)
                phaseC(t)
                if t + 2 < NQ:
                    phaseA(t + 2)

    nc.compile()
    return nc


def _prep_inputs(hidden_states, cos, sin, Wq, Wk, Wv, Wo,
                 conv_q_w, conv_k_w, conv_v_w, q_norm_w, k_norm_w):
    f = np.float32
    bf = ml_dtypes.bfloat16
    x = np.asarray(hidden_states, f)[0]            # [T, D]
    xT = np.ascontiguousarray(x.T.astype(bf))      # [D, T] bf16
    WqT = np.ascontiguousarray(np.asarray(Wq, f).T.astype(bf))
    WkT = np.ascontiguousarray(np.asarray(Wk, f).T.astype(bf))
    WvT = np.ascontiguousarray(np.asarray(Wv, f).T.astype(bf))
    WoT = np.ascontiguousarray(np.asarray(Wo, f).T)

    cosT = np.asarray(cos, f)[0].T                 # [RD, T]
    sinT = np.asarray(sin, f)[0].T
    trig = np.zeros((128, T), f)
    trig[0:RD] = cosT
    # ssin2 block (device rows 0:64): [0:32] = +sin[32:64], [32:64] = -sin[0:32]
    trig[RD:RD + 32] = sinT[32:64]
    trig[RD + 32:2 * RD] = -sinT[0:32]

    # causal mask strip: mask[kl, c] = 1.0 iff kl <= c - 384
    kl = np.arange(128, dtype=f)[:, None]
    cc = np.arange(896, dtype=f)[None, :]
    maskb = (kl <= cc - 384).astype(f)

    nw = np.zeros((128, 2), f)
    nw[:, 0] = np.asarray(q_norm_w, f)
    nw[:, 1] = np.asarray(k_norm_w, f)

    in_maps = []
    for c in range(NCORES):
        sl = slice(c * CPC, (c + 1) * CPC)
        convw = np.zeros((128, 3, 2, KCONV), f)
        for pi, cw in enumerate((conv_q_w, conv_k_w, conv_v_w)):
            convw[:, pi] = np.asarray(cw, f)[sl].reshape(MPC, 128, KCONV
                                                         ).transpose(1, 0, 2)
        in_maps.append({
            "xT": xT,
            "wqT": np.ascontiguousarray(WqT[:, sl]),
            "wkT": np.ascontiguousarray(WkT[:, sl]),
            "wvT": np.ascontiguousarray(WvT[:, sl]),
            "woT": np.ascontiguousarray(WoT[sl, :]),
            "trig": trig,
            "convw": np.ascontiguousarray(convw),
            "normw": nw,
            "maskb": maskb,
        })
    return in_maps


def kernel(hidden_states, cos, sin, Wq, Wk, Wv, Wo,
           conv_q_w, conv_k_w, conv_v_w, q_norm_w, k_norm_w,
           _trace=False):
    global _COMPILED
    if _COMPILED is None:
        _COMPILED = _build()
    nc = _COMPILED
    in_maps = _prep_inputs(hidden_states, cos, sin, Wq, Wk, Wv, Wo,
                           conv_q_w, conv_k_w, conv_v_w, q_norm_w, k_norm_w)
    res = bass_utils.run_bass_kernel_spmd(
        nc, in_maps, core_ids=list(range(NCORES)), trace=_trace)
    acc = np.zeros((D, T), np.float64)
    for r in res.results:
        acc += r["outT"]
    out = np.ascontiguousarray(acc.T.astype(np.float32))[None]
    if _trace:
        kernel._last_results = res
    return out



# revision 15
# speedup vs baseline: 1.2700x; 1.2700x over previous
"""Trainium2 Bass kernel for nn_Attention_34033320854122.

Dense transformer attention block: QKV proj -> causal depthwise conv+SiLU ->
per-head RMSNorm -> partial RoPE -> causal attention -> output projection.

Sharding: tensor-parallel over the 16 heads across 8 NeuronCores (2 heads =
256 channels per core). Each core computes q/k/v for its channels (full
contraction over D), runs attention for its 2 heads, and produces a partial
output projection (outT_partial = Wo[:, cols] @ attn_cols^T). The host sums
the 8 partials and transposes.

Fidelity notes:
- The reference negates the rotated RoPE sub-dim of BOTH q and k; the
  negation cancels in q.k and is skipped.
- softmax without max-subtraction: scores are O(1)-bounded.
- rstd = 1/sqrt(mean(x^2)) computed as exp(-0.5*ln(ms)); eps=1e-5 is
  dropped (ms is O(0.1..1), relative impact < 1e-4).
- norm weights are folded into the RoPE trig tables (rot rows) and a
  per-partition scalar (pass rows); rstd is applied post-rope (it is a
  per-position scalar, commuting with the rotation).

Scheduling: activation table-set switches are minimized (silu-set, then
natural-log/exp set for everything else). RoPE's misaligned half-rotation
products run on GpSimd; V is transposed by the DMA XBAR; per-position
reciprocal-norm rows are partition-broadcast by stride-0 DMA.
"""

from contextlib import ExitStack

import ml_dtypes
import numpy as np

import concourse.bacc as bacc
import concourse.tile as tile
import concourse.mybir as mybir
from concourse import bass_utils

# Problem shape (hardcoded per contract)
B, T, D = 1, 2048, 2048
H, HD = 16, 128
RD = 64
KCONV = 4
NCORES = 8
CPC = D // NCORES      # channels per core = 256
MPC = CPC // HD        # head tiles per core = 2
NT = 512               # free-dim tile for matmuls
NQ = T // NT           # 4 q tiles
KD = D // 128          # 16 contraction chunks
PAD = KCONV - 1        # causal conv history
HT = T // 2            # half-span for conv/silu

F32 = mybir.dt.float32
BF16 = mybir.dt.bfloat16

_COMPILED = None
_DEBUG = False
_DEBUG_RESULTS = None


def _build():
    nc = bacc.Bacc("TRN2", target_bir_lowering=False, debug=False,
                   num_devices=NCORES)

    d = {}
    d["xT"] = nc.dram_tensor("xT", (D, T), BF16, kind="ExternalInput").ap()
    d["wqT"] = nc.dram_tensor("wqT", (D, CPC), BF16, kind="ExternalInput").ap()
    d["wkT"] = nc.dram_tensor("wkT", (D, CPC), BF16, kind="ExternalInput").ap()
    d["wvT"] = nc.dram_tensor("wvT", (D, CPC), BF16, kind="ExternalInput").ap()
    d["woT"] = nc.dram_tensor("woT", (128, MPC, D), BF16,
                              kind="ExternalInput").ap()
    # trig: [:,0]=cos*nwq, [:,1]=cos*nwk, [:,2]=swapped-sin*nwq, [:,3]=..nwk
    d["trig"] = nc.dram_tensor("trig", (64, 4, T), BF16,
                               kind="ExternalInput").ap()
    # per-head norm weights for pass rows: [:,0]=q, [:,1]=k (rows 0:64 == 1)
    d["snw"] = nc.dram_tensor("snw", (128, 2), F32, kind="ExternalInput").ap()
    # conv weights packed [128, proj(3), m(2), tap(4)]
    d["convw"] = nc.dram_tensor("convw", (128, 3, MPC, KCONV), F32,
                                kind="ExternalInput").ap()
    # causal mask strip: mask[kl, j] = 1.0 iff kl <= j - 384
    d["mask4"] = nc.dram_tensor("mask4", (128, 896), BF16,
                                kind="ExternalInput").ap()
    outT = nc.dram_tensor("outT", (D, T), BF16,
                          kind="ExternalOutput").ap()
    dbg = {}
    if _DEBUG:
        dbg["dbg_qf"] = nc.dram_tensor(
            "dbg_qf", (128, MPC, T), BF16, kind="ExternalOutput").ap()
        dbg["dbg_kf"] = nc.dram_tensor(
            "dbg_kf", (128, MPC, T), BF16, kind="ExternalOutput").ap()
        dbg["dbg_vtr"] = nc.dram_tensor(
            "dbg_vtr", (128, MPC, NQ, 4, 128), BF16,
            kind="ExternalOutput").ap()
        dbg["dbg_svq"] = nc.dram_tensor(
            "dbg_svq", (128, MPC, T), BF16, kind="ExternalOutput").ap()
        dbg["dbg_rawq"] = nc.dram_tensor(
            "dbg_rawq", (128, MPC, T + PAD), BF16,
            kind="ExternalOutput").ap()

    inv_sqrt_hd = 1.0 / np.sqrt(HD)

    with ExitStack() as stk:
        tc = stk.enter_context(tile.TileContext(nc))
        if True:
            consts = stk.enter_context(tc.tile_pool(name="consts", bufs=1))
            rawp = stk.enter_context(tc.tile_pool(name="raw", bufs=1))
            svp = stk.enter_context(tc.tile_pool(name="sv", bufs=1))
            finp = stk.enter_context(tc.tile_pool(name="fin", bufs=1))
            wop = stk.enter_context(tc.tile_pool(name="wo", bufs=1))
            psacc = stk.enter_context(
                tc.tile_pool(name="psacc", bufs=4, space="PSUM"))
            pssum = stk.enter_context(
                tc.tile_pool(name="pssum", bufs=1, space="PSUM"))
            pssm = stk.enter_context(
                tc.tile_pool(name="pssm", bufs=3, space="PSUM"))
            # ---- constants ----
            trig_t = consts.tile([64, 4, T], BF16)
            nc.sync.dma_start(trig_t, d["trig"])
            mask4_t = consts.tile([128, 896], BF16)
            nc.scalar.dma_start(mask4_t, d["mask4"])
            convw_t = consts.tile([128, 3, MPC, KCONV], F32)
            nc.sync.dma_start(convw_t, d["convw"])
            snw_t = consts.tile([128, 2], F32)
            nc.scalar.dma_start(snw_t, d["snw"])
            ones_hd = consts.tile([128, 1], BF16)
            nc.vector.memset(ones_hd, 1.0)
            woT_t = wop.tile([128, MPC, D], BF16)
            nc.sync.dma_start(woT_t, d["woT"])

            # ---- persistent buffers ----
            rawq = rawp.tile([128, MPC, T + PAD], BF16)
            rawk = rawp.tile([128, MPC, T + PAD], BF16)
            rawv = rawp.tile([128, MPC, T + PAD], BF16)
            for r in (rawq, rawk, rawv):
                nc.vector.memset(r[:, :, 0:PAD], 0.0)
            raws = (rawq, rawk, rawv)
            # silu outputs (q/k get roped in place; v feeds the transpose)
            svq = svp.tile([128, MPC, T], BF16)
            svk = svp.tile([128, MPC, T], BF16)
            vv = svp.tile([128, MPC, T], BF16)
            svs = (svq, svk, vv)
            # final q/k in head-transposed layout [HD, m, T]
            qfT = finp.tile([128, MPC, T], BF16)
            kfT = finp.tile([128, MPC, T], BF16)
            fins = (qfT, kfT)
            # v^T per 512-block, stride-4 interleave: t = 512*b + 4*p + c
            vtr = finp.tile([128, MPC, NQ, 4, 128], BF16)

            groups = [(0, 0), (0, 1), (1, 0), (1, 1), (2, 0), (2, 1)]

            wqkvp = stk.enter_context(tc.tile_pool(name="wqkv", bufs=1))
            xp = stk.enter_context(tc.tile_pool(name="xb", bufs=2))
            convp = stk.enter_context(tc.tile_pool(name="conv", bufs=4))
            sqp = stk.enter_context(tc.tile_pool(name="sq", bufs=4))
            spp = stk.enter_context(tc.tile_pool(name="sp", bufs=2))
            rrp = stk.enter_context(tc.tile_pool(name="rrb", bufs=2))
            rbcp = stk.enter_context(tc.tile_pool(name="rbc", bufs=2))
            expp = stk.enter_context(tc.tile_pool(name="exp", bufs=3))
            attnp = stk.enter_context(tc.tile_pool(name="attn", bufs=2))
            ostp = stk.enter_context(tc.tile_pool(name="ostage", bufs=3))
            smp = stk.enter_context(tc.tile_pool(name="small", bufs=2))
            if True:
                w_all = wqkvp.tile([128, KD, 3, CPC], BF16)

                def phaseA_loads(tq, first=False):
                    xb = xp.tile([128, KD, NT], BF16, name="xb", tag="xb")
                    for k in range(KD):
                        if first:
                            for pi, wd in enumerate((d["wqT"], d["wkT"],
                                                     d["wvT"])):
                                deng = nc.sync if (k * 3 + pi) % 2 == 0 \
                                    else nc.scalar
                                deng.dma_start(
                                    w_all[:, k, pi, :],
                                    wd[k * 128:(k + 1) * 128, :])
                        deng = (nc.sync if k % 2 == 0 else nc.scalar) \
                            if tq < 2 else nc.sync
                        deng.dma_start(
                            xb[:, k, :],
                            d["xT"][k * 128:(k + 1) * 128,
                                    tq * NT:(tq + 1) * NT])
                    return xb

                def phaseA_mms(tq, xb, drain_eng):
                    # 6 simultaneous accumulations (3 psacc + 3 pssm banks)
                    pst = [psacc.tile([128, NT], F32, tag="acc",
                                      name=f"accA{gi}") for gi in range(3)] \
                        + [pssm.tile([128, NT], F32, tag="sm",
                                     name=f"accB{gi}") for gi in range(3)]
                    for k in range(KD):
                        for gi, (pi, m) in enumerate(groups):
                            nc.tensor.matmul(
                                pst[gi],
                                w_all[:, k, pi, m * 128:(m + 1) * 128],
                                xb[:, k, :],
                                start=(k == 0), stop=(k == KD - 1))
                    for gi, (pi, m) in enumerate(groups):
                        dst = raws[pi][:, m,
                                       PAD + tq * NT:PAD + (tq + 1) * NT]
                        if drain_eng == "v":
                            nc.vector.tensor_copy(dst, pst[gi])
                        else:
                            nc.scalar.activation(
                                dst, pst[gi],
                                mybir.ActivationFunctionType.Copy)

                def conv_silu_sq(pi, m, h, sqtiles):
                    """conv + silu (+square for q/k) on half h."""
                    base = h * HT
                    raw = raws[pi]
                    t0 = convp.tile([128, HT], BF16, tag="cvA", name="cv0")
                    nc.vector.tensor_scalar_mul(
                        t0, raw[:, m, base:base + HT],
                        convw_t[:, pi, m, 0:1])
                    for j in (1, 2, 3):
                        t1 = convp.tile([128, HT], BF16,
                                        tag=("cvB", "cvA")[j % 2], name="cvj")
                        nc.vector.scalar_tensor_tensor(
                            t1, raw[:, m, base + j:base + j + HT],
                            convw_t[:, pi, m, j:j + 1], t0,
                            mybir.AluOpType.mult, mybir.AluOpType.add)
                        t0 = t1
                    sv = svs[pi]
                    nc.scalar.activation(
                        sv[:, m, base:base + HT], t0,
                        mybir.ActivationFunctionType.Silu)
                    if pi < 2:
                        sq = sqp.tile([128, HT], BF16, tag="sq")
                        nc.scalar.activation(
                            sq, sv[:, m, base:base + HT],
                            mybir.ActivationFunctionType.Square,
                            scale=inv_sqrt_hd)
                        sqtiles[(pi, m, h)] = sq

                def phaseBh(h, sqtiles):
                    for m in range(MPC):
                        for pi in range(3):
                            conv_silu_sq(pi, m, h, sqtiles)

                def phaseBs(s, sqtiles):
                    """Finalize slice s: rms-norm rstd, rope, -> qfT/kfT."""
                    sl = slice(s * NT, (s + 1) * NT)
                    h = s // 2
                    c2 = s % 2
                    for m in range(MPC):
                        for pi in range(2):
                            sv = svs[pi][:, m, sl]
                            sq = sqtiles[(pi, m, h)]
                            # ms = sum((x/sqrt(HD))^2) over channels
                            ps_ss = pssm.tile([1, NT], F32, tag="sm",
                                              name="ps_ss")
                            nc.tensor.matmul(
                                ps_ss, ones_hd,
                                sq[:, c2 * NT:(c2 + 1) * NT],
                                start=True, stop=True)
                            # rstd = exp(-0.5*ln(ms)) (same act set as Exp)
                            lns = smp.tile([1, NT], F32, tag="lns",
                                           name="lns")
                            nc.scalar.activation(
                                lns, ps_ss, mybir.ActivationFunctionType.Ln)
                            rrb = rrp.tile([1, NT], BF16, tag="rrb",
                                           name="rrb")
                            nc.scalar.activation(
                                rrb, lns, mybir.ActivationFunctionType.Exp,
                                scale=-0.5)
                            # partition-broadcast via stride-0 DMA
                            rbc = rbcp.tile([128, NT], BF16, tag="rbc",
                                            name="rbc")
                            nc.gpsimd.partition_broadcast(rbc, rrb)
                            # rope: swapped sin products on gpsimd
                            sp = spp.tile([64, NT], BF16, tag="sp",
                                          name="sp")
                            nc.gpsimd.tensor_mul(
                                sp[0:32, :], sv[32:64, :],
                                trig_t[32:64, 2 + pi, sl])
                            nc.gpsimd.tensor_mul(
                                sp[32:64, :], sv[0:32, :],
                                trig_t[0:32, 2 + pi, sl])
                            # cos product in place, then add sin part
                            nc.vector.tensor_mul(
                                sv[0:64, :], sv[0:64, :],
                                trig_t[0:64, pi, sl])
                            nc.vector.tensor_add(sv[0:64, :], sv[0:64, :],
                                                 sp)
                            # fin = sv * snw * rstd
                            nc.vector.scalar_tensor_tensor(
                                fins[pi][:, m, sl], sv,
                                snw_t[:, pi:pi + 1], rbc,
                                mybir.AluOpType.mult, mybir.AluOpType.mult)

                def v_transpose(b):
                    for m in range(MPC):
                        nc.sync.dma_start_transpose(
                            vtr[:, m, b],
                            vv[:, m, b * NT:(b + 1) * NT])

                def phaseC(t, interleave=None):
                    qsl = slice(t * NT, (t + 1) * NT)
                    nch = 4 * (t + 1)
                    attn_m = []
                    for m in range(MPC):
                        ps_attn = psacc.tile([128, NT], F32, tag="acc",
                                             name="ps_attn")
                        ps_sum = pssum.tile([1, NT], F32, tag="sum1",
                                            name="ps_sum")

                        def qk(kc):
                            ps_s = pssm.tile([128, NT], F32, tag="sm",
                                             name="ps_s")
                            nc.tensor.matmul(
                                ps_s,
                                kfT[:, m, kc * 128:(kc + 1) * 128],
                                qfT[:, m, qsl], start=True, stop=True)
                            e = expp.tile([128, NT], BF16, tag="e", name="e")
                            nc.scalar.activation(
                                e, ps_s, mybir.ActivationFunctionType.Exp,
                                scale=inv_sqrt_hd)
                            dd = kc * 128 - t * NT
                            if dd >= 0:  # diagonal chunk: causal mask
                                nc.vector.tensor_mul(
                                    e, e, mask4_t[:, 384 - dd:896 - dd])
                            return e

                        epipe = [qk(kc) for kc in range(min(2, nch))]
                        for kc in range(nch):
                            if kc + 2 < nch:
                                epipe.append(qk(kc + 2))
                            e = epipe.pop(0)
                            b, c = kc // 4, kc % 4
                            nc.tensor.matmul(
                                ps_attn, vtr[:, m, b, c, :], e,
                                start=(kc == 0), stop=(kc == nch - 1))
                            nc.tensor.matmul(
                                ps_sum, ones_hd, e,
                                start=(kc == 0), stop=(kc == nch - 1))
                        # normalize by 1/sumexp via stride-0 DMA broadcast
                        rrf = smp.tile([1, NT], F32, tag="rrf", name="rrf")
                        nc.vector.reciprocal_approx_fast(rrf, ps_sum)
                        rrc = smp.tile([1, NT], BF16, tag="rrc", name="rrc")
                        nc.vector.tensor_copy(rrc, rrf)
                        rbc = rbcp.tile([128, NT], BF16, tag="rbc",
                                        name="rbcC")
                        nc.gpsimd.partition_broadcast(rbc, rrc)
                        am = attnp.tile([128, NT], BF16, tag="am", name="am")
                        nc.vector.tensor_mul(am, ps_attn, rbc)
                        attn_m.append(am)
                        if interleave:
                            interleave.pop(0)()
                    # output projection (wo resident)
                    for i in range(KD):
                        ps_o = psacc.tile([128, NT], F32, tag="acc",
                                          name="ps_o")
                        for j in range(MPC):
                            nc.tensor.matmul(
                                ps_o, woT_t[:, j, i * 128:(i + 1) * 128],
                                attn_m[j], start=(j == 0),
                                stop=(j == MPC - 1))
                        ost = ostp.tile([128, NT], BF16, tag="ost",
                                        name="ost")
                        if i % 2 == 0:
                            nc.vector.tensor_copy(ost, ps_o)
                        else:
                            nc.scalar.activation(
                                ost, ps_o, mybir.ActivationFunctionType.Copy)
                        deng = nc.sync if i % 2 == 0 else nc.gpsimd
                        deng.dma_start(outT[i * 128:(i + 1) * 128, qsl],
                                       ost)
                        if interleave:
                            interleave.pop(0)()

                # ================= emission schedule =================
                sqtiles = {}
                xb0 = phaseA_loads(0, first=True)
                xb1 = phaseA_loads(1)
                phaseA_mms(0, xb0, "v")
                phaseA_mms(1, xb1, "v")
                xb2 = phaseA_loads(2)
                xb3 = phaseA_loads(3)
                phaseA_mms(2, xb2, "s")
                phaseBh(0, sqtiles)       # conv/silu/sq for t in [0, 1024)
                phaseA_mms(3, xb3, "s")
                phaseBs(0, sqtiles)
                phaseBs(1, sqtiles)
                v_transpose(0)
                v_transpose(1)

                # Bh1 pieces interleaved into C0/C1 emission
                pieces = []
                for m in range(MPC):
                    for pi in range(3):
                        pieces.append(
                            lambda pi=pi, m=m: conv_silu_sq(pi, m, 1,
                                                            sqtiles))
                nfill = 2 * MPC + 2 * KD  # interleave slots in C0+C1
                while len(pieces) < nfill:
                    pieces.append(lambda: None)
                phaseC(0, interleave=pieces)
                phaseC(1, interleave=pieces)
                assert not pieces
                phaseBs(2, sqtiles)
                phaseBs(3, sqtiles)
                v_transpose(2)
                v_transpose(3)
                phaseC(2)
                phaseC(3)
                if _DEBUG:
                    nc.sync.dma_start(dbg["dbg_qf"], qfT)
                    nc.sync.dma_start(dbg["dbg_kf"], kfT)
                    nc.sync.dma_start(dbg["dbg_vtr"], vtr)
                    nc.sync.dma_start(dbg["dbg_svq"], svq)
                    nc.sync.dma_start(dbg["dbg_rawq"], rawq)

    nc.compile()
    return nc


def _prep_inputs(hidden_states, cos, sin, Wq, Wk, Wv, Wo,
                 conv_q_w, conv_k_w, conv_v_w, q_norm_w, k_norm_w):
    f = np.float32
    bf = ml_dtypes.bfloat16
    x = np.asarray(hidden_states, f)[0]            # [T, D]
    xT = np.ascontiguousarray(x.T.astype(bf))      # [D, T] bf16
    WqT = np.ascontiguousarray(np.asarray(Wq, f).T.astype(bf))
    WkT = np.ascontiguousarray(np.asarray(Wk, f).T.astype(bf))
    WvT = np.ascontiguousarray(np.asarray(Wv, f).T.astype(bf))
    WoT = np.asarray(Wo, f).T                      # [CPC(full D), D]

    cosT = np.asarray(cos, f)[0].T                 # [RD, T]
    sinT = np.asarray(sin, f)[0].T
    nwq = np.asarray(q_norm_w, f)
    nwk = np.asarray(k_norm_w, f)

    # trig tables with norm weights folded into the rotary rows.
    # sin table indexed by SOURCE row r (out row p = r xor 32):
    #   r in 0:32  -> p = r+32: +sin[p]*nw[p]
    #   r in 32:64 -> p = r-32: -sin[p]*nw[p]
    def mk_trig(nw):
        cosb = cosT * nw[0:RD, None]
        ss = np.zeros((RD, T), f)
        ss[0:32] = sinT[32:64] * nw[32:64, None]
        ss[32:64] = -sinT[0:32] * nw[0:32, None]
        return cosb, ss

    cosq, ssq = mk_trig(nwq)
    cosk, ssk = mk_trig(nwk)
    trig = np.stack([cosq, cosk, ssq, ssk], axis=1).astype(bf)  # [64,4,T]

    snw = np.ones((128, 2), f)
    snw[RD:128, 0] = nwq[RD:128]
    snw[RD:128, 1] = nwk[RD:128]

    # causal mask strip: mask[kl, j] = 1.0 iff kl <= j - 384
    pp = np.arange(128, dtype=f)[:, None]
    jj = np.arange(896, dtype=f)[None, :]
    mask4 = (pp <= jj - 384).astype(bf)

    in_maps = []
    for ci in range(NCORES):
        sl = slice(ci * CPC, (ci + 1) * CPC)
        convw = np.zeros((128, 3, MPC, KCONV), f)
        for pi, cw in enumerate((conv_q_w, conv_k_w, conv_v_w)):
            convw[:, pi] = np.asarray(cw, f)[sl].reshape(MPC, 128, KCONV
                                                         ).transpose(1, 0, 2)
        wo_res = np.ascontiguousarray(
            WoT[sl].reshape(MPC, 128, D).transpose(1, 0, 2).astype(bf))
        in_maps.append({
            "xT": xT,
            "wqT": np.ascontiguousarray(WqT[:, sl]),
            "wkT": np.ascontiguousarray(WkT[:, sl]),
            "wvT": np.ascontiguousarray(WvT[:, sl]),
            "woT": wo_res,
            "trig": trig,
            "snw": snw,
            "convw": np.ascontiguousarray(convw),
            "mask4": np.ascontiguousarray(mask4),
        })
    return in_maps


def kernel(hidden_states, cos, sin, Wq, Wk, Wv, Wo,
           conv_q_w, conv_k_w, conv_v_w, q_norm_w, k_norm_w,
           _trace=False):
    global _COMPILED
    if _COMPILED is None:
        _COMPILED = _build()
    nc = _COMPILED
    in_maps = _prep_inputs(hidden_states, cos, sin, Wq, Wk, Wv, Wo,
                           conv_q_w, conv_k_w, conv_v_w, q_norm_w, k_norm_w)
    res = bass_utils.run_bass_kernel_spmd(
        nc, in_maps, core_ids=list(range(NCORES)), trace=_trace)
    if _DEBUG:
        global _DEBUG_RESULTS
        _DEBUG_RESULTS = res.results
    acc = np.zeros((D, T), np.float64)
    for r in res.results:
        acc += np.asarray(r["outT"], np.float64)
    out = np.ascontiguousarray(acc.T.astype(np.float32))[None]
    if _trace:
        kernel._last_results = res
    return out
